# revision 14
# baseline (speedup 1.0000x reference)
"""Trainium2 Bass kernel for nn_AttackLinkPredictor (2-layer RGCN + link-pred MLP).

Distribution (8 NeuronCores, one SPMD NEFF):
  - Nodes dst-sharded: core c owns nodes [c*NPC, (c+1)*NPC).
  - Per core, edges are split into 4 src ranges (dma_gather idx is int16 so the
    gather table must stay <32768 rows), sorted by dst, packed into 128-edge
    chunks of whole dst-segments (<=32 distinct dst per chunk).
  - Aggregation: per chunk, gathered messages [128e, 128f] are segment-summed
    via a one-hot matmul (S[e, slot] = (dstloc[e] == slot)) into PSUM
    [32 slots x 128 f] partials, which are dma_scatter_add'ed into a
    node-major agg buffer in DRAM (rows unique per call; races impossible).
  - h = relu(mean @ Wr + h @ Wroot + b) computed feature-major per 128-node
    tile (PE transposes), written node-major, AllGather'ed for the next
    layer's gathers.
  - MLP: pairs sharded by the a-endpoint's owning core (a-gather reads the
    core's own h2 shard); b rows gathered via a 2-stage bounce (4-range
    compact gather -> DRAM -> re-gather in pair order).

Self-contained: hardcodes shapes; host planning is pure numpy.
"""

import sys

for _p in ("/opt/trn_rl_repo", "/opt/trn_rl_repo/concourse"):
    if _p not in sys.path:
        sys.path.insert(0, _p)

import numpy as np

CORES = 8
D = 128
RANGES = 4
SLOTS = 64  # dst segment slots per 128-edge chunk
SENT = 999.0  # dstloc sentinel; never matches iota [0, SLOTS)


class Cfg:
    def __init__(self, n_nodes, n_edges, n_pairs, msg_slab=48):
        assert n_nodes % (CORES * RANGES) == 0
        self.n_nodes = n_nodes
        self.n_edges = n_edges
        self.n_pairs = n_pairs
        self.npc = n_nodes // CORES
        self.rq = n_nodes // RANGES
        assert self.rq < 32768 and self.npc < 32700
        self.wt = -(-self.npc // 128)  # weight tiles
        self.nt = self.wt if self.npc % 128 else self.wt + 1  # +trash tile
        self.agg_rows = self.nt * 128
        self.trash = self.npc
        self.msg_slab = msg_slab  # chunks per gather slab (multiple of 16)
        assert msg_slab % 16 == 0


REAL = Cfg(100000, 600000, 100000)


# ---------------------------------------------------------------------------
# host planning (numpy only)
# ---------------------------------------------------------------------------


def _wrap16(idx, dtype=np.int16):
    """token i -> [i % 16, i // 16], replicated to 128 partitions."""
    idx = np.asarray(idx, dtype)
    assert idx.size % 16 == 0
    w = idx.reshape(-1, 16).T.copy()
    return np.ascontiguousarray(np.tile(w, (8, 1)))


def _pack_chunks(srcl, dl):
    """Edges (any order) -> chunks of <=128 edges / <=SLOTS whole dst-segments.

    Returns list of (gidx[128] i16, slot[128] f32, rows[SLOTS] i32 with -1 pad).
    """
    order = np.argsort(dl, kind="stable")
    srcl = np.asarray(srcl)[order]
    dl = np.asarray(dl)[order]
    n = dl.size
    # segment boundaries
    if n == 0:
        return []
    starts = np.flatnonzero(np.r_[True, dl[1:] != dl[:-1]])
    ends = np.r_[starts[1:], n]
    chunks = []
    si = 0
    nseg = starts.size
    while si < nseg:
        e0 = starts[si]
        sj = si
        while (
            sj < nseg
            and (sj - si) < SLOTS
            and (ends[sj] - e0) <= 128
        ):
            sj += 1
        assert sj > si, f"segment larger than a chunk: {ends[si] - starts[si]}"
        e = ends[sj - 1]
        cnt = e - e0
        g = np.zeros(128, np.int16)
        s = np.full(128, SENT, np.float32)
        rows = np.full(SLOTS, -1, np.int32)
        g[:cnt] = srcl[e0:e]
        seg_vals = dl[starts[si:sj]]
        lut = {int(v): k for k, v in enumerate(seg_vals)}
        s[:cnt] = [lut[int(v)] for v in dl[e0:e]]
        rows[: seg_vals.size] = seg_vals
        chunks.append((g, s, rows))
        si = sj
    return chunks


def _plan_edges(cfg, src_g, dst_g):
    percore = []
    for c in range(CORES):
        m = (dst_g // cfg.npc) == c
        src = src_g[m]
        dstl = dst_g[m] - c * cfg.npc
        by_range = []
        for r in range(RANGES):
            sel = (src // cfg.rq) == r
            by_range.append(_pack_chunks(src[sel] - r * cfg.rq, dstl[sel]))
        percore.append(by_range)

    c_r = []
    for r in range(RANGES):
        m = max(len(percore[c][r]) for c in range(CORES))
        c_r.append(-(-max(m, 16) // 16) * 16)

    dummy = (
        np.zeros(128, np.int16),
        np.full(128, SENT, np.float32),
        np.full(SLOTS, -1, np.int32),
    )
    data = []
    for c in range(CORES):
        gidx_r, slot_cols, scat_tokens = [], [], []
        for r in range(RANGES):
            chunks = percore[c][r] + [dummy] * (c_r[r] - len(percore[c][r]))
            gidx_r.append(_wrap16(np.concatenate([ch[0] for ch in chunks])))
            slot_cols.append(np.stack([ch[1] for ch in chunks], 1))  # [128, C_r]
            rows_all = np.stack([ch[2] for ch in chunks])  # [C_r, SLOTS]
            rows_all = np.where(rows_all < 0, cfg.trash, rows_all)
            for s0 in range(0, c_r[r], cfg.msg_slab):
                ns = min(cfg.msg_slab, c_r[r] - s0)
                toks = np.empty(ns * SLOTS, np.int16)
                t = np.arange(ns * SLOTS)
                col, p = t // 128, t % 128
                chunk = s0 + 8 * (col // 4) + 2 * (col % 4) + p // 64
                toks[:] = rows_all[chunk, p % 64]
                scat_tokens.append(toks)
        data.append(
            dict(
                gidx_r=gidx_r,
                dstloc=np.concatenate(slot_cols, 1),
                scat=_wrap16(np.concatenate(scat_tokens)),
            )
        )
    return data, dict(c_r=c_r, ct=sum(c_r))


def _plan_pairs(cfg, a_g, b_g):
    sel_idx = [np.nonzero((a_g // cfg.npc) == c)[0] for c in range(CORES)]
    pmax = -(-max(max(s.size for s in sel_idx), 512) // 512) * 512

    bmax_r = []
    for r in range(RANGES):
        mx = max(int(((b_g[s] // cfg.rq) == r).sum()) for s in sel_idx)
        bmax_r.append(-(-max(mx, 128) // 128) * 128)
    boff = np.concatenate([[0], np.cumsum(bmax_r)]).astype(np.int64)
    bst_rows = int(boff[-1])
    assert bst_rows < 32768

    data = []
    for c in range(CORES):
        idx = sel_idx[c]
        a = a_g[idx] - c * cfg.npc
        b = b_g[idx]
        a_idx = np.zeros(pmax, np.int16)
        a_idx[: idx.size] = a
        b1, b2 = [], np.zeros(pmax, np.int16)
        for r in range(RANGES):
            s = np.nonzero((b // cfg.rq) == r)[0]
            l1 = np.zeros(bmax_r[r], np.int16)
            l1[: s.size] = b[s] - r * cfg.rq
            b1.append(l1)
            b2[s] = (boff[r] + np.arange(s.size)).astype(np.int16)
        data.append(
            dict(
                a_idx=_wrap16(a_idx),
                b1_idx=_wrap16(np.concatenate(b1)),
                b2_idx=_wrap16(b2),
                sel=idx,
            )
        )
    return data, dict(pmax=pmax, bmax_r=bmax_r, bst_rows=bst_rows)


def plan(cfg, x, edge_index, edge_pairs):
    src_g = np.asarray(edge_index[0], np.int64)
    dst_g = np.asarray(edge_index[1], np.int64)
    edata, edims = _plan_edges(cfg, src_g, dst_g)
    pdata, pdims = _plan_pairs(
        cfg,
        np.asarray(edge_pairs[:, 0], np.int64),
        np.asarray(edge_pairs[:, 1], np.int64),
    )

    x = np.asarray(x, np.float32)
    in_maps = []
    for c in range(CORES):
        cnt = np.bincount(
            dst_g[(dst_g // cfg.npc) == c] - c * cfg.npc, minlength=cfg.agg_rows
        ).astype(np.float32)
        r = (1.0 / np.maximum(cnt, 1.0)).astype(np.float32)
        r[cfg.npc :] = 0.0

        xT = np.zeros((128, cfg.wt * 128), np.float32)
        sl = x[c * cfg.npc : (c + 1) * cfg.npc]
        xT[:, : sl.shape[0]] = sl.T

        m = dict(
            x=x,
            dstloc=np.ascontiguousarray(edata[c]["dstloc"].astype(np.float32)),
            scat=edata[c]["scat"],
            recip=np.ascontiguousarray(r.reshape(cfg.nt, 128).T),
            iota=np.ascontiguousarray(
                np.tile(np.arange(SLOTS, dtype=np.float32)[None, None, :], (128, 16, 1))
            ),
            ident=np.eye(128, dtype=np.float32),
            xmineT=xT,
            a_idx=pdata[c]["a_idx"],
            b1_idx=pdata[c]["b1_idx"],
            b2_idx=pdata[c]["b2_idx"],
        )
        for r_ in range(RANGES):
            m[f"gidx{r_}"] = edata[c]["gidx_r"][r_]
        in_maps.append(m)
    dims = dict(edims, **pdims)
    return in_maps, dims, pdata


# ---------------------------------------------------------------------------
# bass program
# ---------------------------------------------------------------------------


def build(cfg, dims, stages=5):
    import concourse.bacc as bacc
    import concourse.mybir as mybir
    from concourse import tile

    f32 = mybir.dt.float32
    i16 = mybir.dt.int16
    AT = mybir.ActivationFunctionType
    OP = mybir.AluOpType

    nc = bacc.Bacc("TRN2", target_bir_lowering=False)
    c_r, ct = dims["c_r"], dims["ct"]
    pmax, bst_rows = dims["pmax"], dims["bst_rows"]
    NPC, RQ, NT, WT, AGG = cfg.npc, cfg.rq, cfg.nt, cfg.wt, cfg.agg_rows
    SLAB = cfg.msg_slab
    WTG = -(-WT // 4)  # supertile groups of 4 node-tiles

    def param(name, shape, dt=f32):
        return nc.declare_dram_parameter(name, shape, dt, isOutput=False)

    x = param("x", [cfg.n_nodes, D])
    Wr_p = [param("Wr1", [D, D]), param("Wr2", [D, D])]
    Wroot_p = [param("Wroot1", [D, D]), param("Wroot2", [D, D])]
    b_p = [param("b1", [D]), param("b2", [D])]
    Wp1 = param("Wp1", [2 * D, D])
    bp1 = param("bp1", [D])
    Wp2 = param("Wp2", [D, 64])
    bp2 = param("bp2", [64])
    Wp3 = param("Wp3", [64, 1])
    bp3 = param("bp3", [1])
    gidx = [param(f"gidx{r}", [128, c_r[r] * 8], i16) for r in range(RANGES)]
    dstloc = param("dstloc", [128, ct])
    scat = param("scat", [128, ct * 4], i16)
    recip = param("recip", [128, NT])
    iota_p = param("iota", [128, 16, SLOTS])
    ident_p = param("ident", [128, 128])
    xmineT = param("xmineT", [128, WT * 128])
    a_idx = param("a_idx", [128, pmax // 16], i16)
    b1_idx = param("b1_idx", [128, bst_rows // 16], i16)
    b2_idx = param("b2_idx", [128, pmax // 16], i16)
    p_out = nc.declare_dram_parameter("p_out", [1, pmax], f32, isOutput=True)

    agg = nc.dram_tensor("agg", [AGG, D], f32)
    h_mine = [nc.dram_tensor(f"h{l}_mine", [AGG, D], f32) for l in (1, 2)]
    h_full = [
        nc.dram_tensor(f"h{l}_full", [cfg.n_nodes, D], f32, addr_space="Shared")
        for l in (1, 2)
    ]
    bstage = nc.dram_tensor("bstage", [bst_rows, D], f32)
    rg = [list(range(CORES))]

    with tile.TileContext(nc) as tc:
        with (
            tc.tile_pool(name="const", bufs=1) as cpool,
            tc.tile_pool(name="msg", bufs=2) as mpool,
            tc.tile_pool(name="gi", bufs=2) as gip,
            tc.tile_pool(name="sbuild", bufs=4) as spool,
            tc.tile_pool(name="stg", bufs=2) as stg,
            tc.tile_pool(name="scidx", bufs=2) as scp,
            tc.tile_pool(name="wt", bufs=2) as wpool,
            tc.tile_pool(name="arena", bufs=WTG) as arena,
            tc.tile_pool(name="ps", bufs=2, space="PSUM") as psp,
        ):

            _cseq = [0]

            def cload(shape, src_ap, dt=f32):
                _cseq[0] += 1
                t = cpool.tile(shape, dt, tag=f"const{_cseq[0]}")
                nc.sync.dma_start(out=t[:], in_=src_ap)
                return t

            iota_t = cload([128, 16, SLOTS], iota_p[:])
            ident_t = cload([128, 128], ident_p[:])
            dstloc_t = cload([128, ct], dstloc[:, :])
            recip_t = cload([128, NT], recip[:, :])
            Wr_t = [cload([128, 128], w[:, :]) for w in Wr_p]
            Wroot_t = [cload([128, 128], w[:, :]) for w in Wroot_p]
            b_t = [cload([128, 1], b[:].unsqueeze(1)) for b in b_p]
            Wp1a_t = cload([128, 128], Wp1[0:128, :])
            Wp1b_t = cload([128, 128], Wp1[128:256, :])
            bp1_t = cload([128, 1], bp1[:].unsqueeze(1))
            Wp2_t = cload([128, 64], Wp2[:, :])
            bp2_t = cload([64, 1], bp2[:].unsqueeze(1))
            Wp3_t = cload([64, 1], Wp3[:, :])
            bp3_t = cload([1, 1], bp3[:].unsqueeze(1))

            zt = cpool.tile([128, 512], f32)
            nc.vector.memset(zt[:], 0.0)

            def zero_dram(t, rows):
                zview = zt[:].rearrange("p (c d) -> p c d", d=D)
                for r0 in range(0, rows, 512):
                    n = min(512, rows - r0)
                    assert n % 128 == 0
                    nc.sync.dma_start(
                        out=t[r0 : r0 + n, :].rearrange("(c p) d -> p c d", p=128),
                        in_=zview[:, : n // 128, :],
                    )

            # xT arena tiles for layer-1 root term (host pre-transposed)
            xT_tiles = []
            for g0 in range(0, WT, 4):
                ng = min(4, WT - g0)
                t = arena.tile([128, 512], f32, tag="arena")
                nc.sync.dma_start(
                    out=t[:, : ng * 128], in_=xmineT[:, g0 * 128 : (g0 + ng) * 128]
                )
                xT_tiles.append(t)

            def gather_slabs(table_ap, idx_param_ap, n_tokens, cb):
                """Gather n_tokens rows in SLAB*128-token slabs; cb(m, s0, ns)."""
                for s0 in range(0, n_tokens // 128, SLAB):
                    ns = min(SLAB, n_tokens // 128 - s0)
                    gi = gip.tile([128, SLAB * 8], i16, tag="gi")
                    nc.sync.dma_start(
                        out=gi[:, : ns * 8], in_=idx_param_ap[:, s0 * 8 : (s0 + ns) * 8]
                    )
                    m = mpool.tile([128, SLAB, 128], f32, tag="msg")
                    nc.gpsimd.dma_gather(
                        out_ap=m[:, :ns, :],
                        in_ap=table_ap,
                        idxs_ap=gi[:, : ns * 8],
                        num_idxs=ns * 128,
                        num_idxs_reg=ns * 128,
                        elem_size=D,
                        single_packet=False,
                    )
                    cb(m, s0, ns)

            def layer(li, table, root_tiles, out_mine, out_full, stages=5):
                zero_dram(agg, AGG)
                gchunk_base = [0]
                tok_base = [0]
                for r in range(RANGES):

                    def agg_slab(m, s0, ns, r=r):
                        gchunk = gchunk_base[0] + s0
                        st = stg.tile([128, SLAB // 2, 128], f32, tag="stg")
                        for b in range(ns // 16):
                            S = spool.tile([128, 16, SLOTS], f32, tag="S")
                            dl = dstloc_t[:, gchunk + b * 16 : gchunk + (b + 1) * 16]
                            nc.vector.tensor_tensor(
                                S[:],
                                dl.unsqueeze(2).broadcast_to([128, 16, SLOTS]),
                                iota_t[:],
                                OP.is_equal,
                            )
                            for half in range(2):
                                ps = psp.tile([128, 512], f32, tag="A")
                                for j in range(8):
                                    nc.tensor.matmul(
                                        ps[
                                            (j % 2) * 64 : (j % 2) * 64 + 64,
                                            (j // 2) * 128 : (j // 2) * 128 + 128,
                                        ],
                                        S[:, half * 8 + j, :],
                                        m[:, b * 16 + half * 8 + j, :],
                                        start=True,
                                        stop=True,
                                        tile_position=(0, (j % 2) * 64),
                                    )
                                nc.vector.tensor_copy(
                                    st[:, (b * 2 + half) * 4 : (b * 2 + half) * 4 + 4, :],
                                    ps[:].rearrange("p (c d) -> p c d", d=128),
                                )
                        tok = tok_base[0] + s0 * SLOTS
                        sc = scp.tile([128, SLAB * 4], i16, tag="sc")
                        nc.sync.dma_start(
                            out=sc[:, : ns * 4],
                            in_=scat[:, tok // 16 : (tok + ns * SLOTS) // 16],
                        )
                        nc.gpsimd.dma_scatter_add(
                            agg[:, :],
                            st[:, : ns * SLOTS // 128, :],
                            sc[:, : ns * 4],
                            num_idxs=ns * SLOTS,
                            num_idxs_reg=ns * SLOTS,
                            elem_size=D,
                            single_packet=False,
                        )

                    gather_slabs(
                        table[r * RQ : (r + 1) * RQ, :], gidx[r][:, :], c_r[r] * 128, agg_slab
                    )
                    gchunk_base[0] += c_r[r]
                    tok_base[0] += c_r[r] * SLOTS

                # dense stage
                out_arena = []
                if stages < 2:
                    return out_arena
                for gi_, g0 in enumerate(range(0, WT, 4)):
                    ng = min(4, WT - g0)
                    pT = psp.tile([128, 512], f32, tag="T")
                    for k in range(ng):
                        t = g0 + k
                        at = wpool.tile([128, 128], f32, tag="load")
                        nc.sync.dma_start(out=at[:], in_=agg[t * 128 : (t + 1) * 128, :])
                        asc = wpool.tile([128, 128], f32, tag="scaled")
                        nc.vector.tensor_scalar(
                            asc[:], at[:], recip_t[:, t : t + 1], None, op0=OP.mult
                        )
                        nc.tensor.transpose(
                            pT[:, k * 128 : (k + 1) * 128], asc[:], ident_t[:]
                        )
                    mean_t = wpool.tile([128, 512], f32, tag="meanT")
                    nc.scalar.copy(mean_t[:, : ng * 128], pT[:, : ng * 128])
                    pH = psp.tile([128, 512], f32, tag="H")
                    nc.tensor.matmul(
                        pH[:, : ng * 128], Wr_t[li][:], mean_t[:, : ng * 128], start=True, stop=False
                    )
                    nc.tensor.matmul(
                        pH[:, : ng * 128],
                        Wroot_t[li][:],
                        root_tiles[gi_][:, : ng * 128],
                        start=False,
                        stop=True,
                    )
                    if li == 0:
                        hT = arena.tile([128, 512], f32, tag="arena")
                        out_arena.append(hT)
                    else:
                        hT = wpool.tile([128, 512], f32, tag="hT")
                    nc.scalar.activation(
                        hT[:, : ng * 128], pH[:, : ng * 128], AT.Relu, bias=b_t[li][:]
                    )
                    pO = psp.tile([128, 512], f32, tag="O")
                    for k in range(ng):
                        nc.tensor.transpose(
                            pO[:, k * 128 : (k + 1) * 128],
                            hT[:, k * 128 : (k + 1) * 128],
                            ident_t[:],
                        )
                    hn = wpool.tile([128, 512], f32, tag="hnode")
                    nc.scalar.copy(hn[:, : ng * 128], pO[:, : ng * 128])
                    nc.sync.dma_start(
                        out=out_mine[g0 * 128 : g0 * 128 + ng * 128, :].rearrange(
                            "(c p) d -> p c d", p=128
                        ),
                        in_=hn[:, : ng * 128].rearrange("p (c d) -> p c d", d=128),
                    )
                if stages >= 3:
                    nc.gpsimd.collective_compute(
                        "AllGather",
                        OP.bypass,
                        replica_groups=rg,
                        ins=[out_mine[0:NPC, :].opt()],
                        outs=[out_full[:, :].opt()],
                    )
                return out_arena

            h1T = layer(0, x, xT_tiles, h_mine[0], h_full[0], stages)
            if stages >= 4:
                layer(1, h_full[0], h1T, h_mine[1], h_full[1], stages)

            # ---- MLP stage ----
            if stages < 5:
                return_early = True
            else:
                return_early = False
            boff = 0
            for r in range(RANGES if not return_early else 0):
                nrows = dims["bmax_r"][r]

                def to_bstage(m, s0, ns, boff=boff):
                    nc.sync.dma_start(
                        out=bstage[
                            boff + s0 * 128 : boff + (s0 + ns) * 128, :
                        ].rearrange("(c p) d -> p c d", p=128),
                        in_=m[:, :ns, :],
                    )

                gather_slabs(
                    h_full[1][r * RQ : (r + 1) * RQ, :],
                    b1_idx[:, boff * 8 // 128 :],
                    nrows,
                    to_bstage,
                )
                boff += nrows

            def mlp_slab(mA, s0, ns):
                # second gather for this slab's b rows, then process 512-groups
                giB = gip.tile([128, SLAB * 8], i16, tag="gi")
                nc.sync.dma_start(
                    out=giB[:, : ns * 8], in_=b2_idx[:, s0 * 8 : (s0 + ns) * 8]
                )
                mB = mpool.tile([128, SLAB, 128], f32, tag="msg")
                nc.gpsimd.dma_gather(
                    out_ap=mB[:, :ns, :],
                    in_ap=bstage[:, :],
                    idxs_ap=giB[:, : ns * 8],
                    num_idxs=ns * 128,
                    num_idxs_reg=ns * 128,
                    elem_size=D,
                    single_packet=False,
                )
                for q0 in range(0, ns, 4):
                    nq = min(4, ns - q0)
                    pEa = psp.tile([128, 512], f32, tag="T")
                    pEb = psp.tile([128, 512], f32, tag="O")
                    for k in range(nq):
                        nc.tensor.transpose(
                            pEa[:, k * 128 : (k + 1) * 128], mA[:, q0 + k, :], ident_t[:]
                        )
                        nc.tensor.transpose(
                            pEb[:, k * 128 : (k + 1) * 128], mB[:, q0 + k, :], ident_t[:]
                        )
                    eTa = wpool.tile([128, 512], f32, tag="eTa")
                    eTb = wpool.tile([128, 512], f32, tag="eTb")
                    nc.scalar.copy(eTa[:, : nq * 128], pEa[:, : nq * 128])
                    nc.scalar.copy(eTb[:, : nq * 128], pEb[:, : nq * 128])
                    w = nq * 128
                    pZ1 = psp.tile([128, 512], f32, tag="H")
                    nc.tensor.matmul(pZ1[:, :w], Wp1a_t[:], eTa[:, :w], start=True, stop=False)
                    nc.tensor.matmul(pZ1[:, :w], Wp1b_t[:], eTb[:, :w], start=False, stop=True)
                    z1 = wpool.tile([128, 512], f32, tag="z1")
                    nc.scalar.activation(z1[:, :w], pZ1[:, :w], AT.Relu, bias=bp1_t[:])
                    pZ2 = psp.tile([128, 512], f32, tag="A")
                    nc.tensor.matmul(pZ2[0:64, :w], Wp2_t[:], z1[:, :w], start=True, stop=True)
                    z2 = wpool.tile([64, 512], f32, tag="z2")
                    nc.scalar.activation(z2[:, :w], pZ2[0:64, :w], AT.Relu, bias=bp2_t[:])
                    pZ3 = psp.tile([128, 512], f32, tag="T")
                    nc.tensor.matmul(pZ3[0:1, :w], Wp3_t[:], z2[:, :w], start=True, stop=True)
                    og = wpool.tile([1, 512], f32, tag="outg")
                    nc.scalar.activation(
                        og[0:1, :w], pZ3[0:1, :w], AT.Sigmoid, bias=bp3_t[:]
                    )
                    nc.sync.dma_start(
                        out=p_out[0:1, (s0 + q0) * 128 : (s0 + q0) * 128 + w],
                        in_=og[0:1, :w],
                    )

            if not return_early:
                gather_slabs(h_mine[1][0 : WT * 128, :], a_idx[:, :], pmax, mlp_slab)

    nc.compile()
    return nc


# ---------------------------------------------------------------------------
# entry point
# ---------------------------------------------------------------------------


def _run_spmd(nc, in_maps, trace=False, tmpdir=None):
    from concourse import bass_utils

    return bass_utils.run_bass_kernel_spmd(
        nc, in_maps, core_ids=list(range(CORES)), trace=trace, tmpdir=tmpdir
    )


def prepare(cfg, inputs):
    """plan + assemble per-core in_maps (without building the bass program)."""
    weights = {
        k: np.asarray(inputs[k], np.float32)
        for k in (
            "Wr1",
            "Wroot1",
            "b1",
            "Wr2",
            "Wroot2",
            "b2",
            "Wp1",
            "bp1",
            "Wp2",
            "bp2",
            "Wp3",
            "bp3",
        )
    }
    in_maps, dims, pdata = plan(
        cfg, inputs["x"], np.asarray(inputs["edge_index"]), np.asarray(inputs["edge_pairs"])
    )
    for m in in_maps:
        m.update(weights)
    return in_maps, dims, pdata


def unshard(cfg, results, pdata):
    out = np.empty(cfg.n_pairs, np.float32)
    for c in range(CORES):
        sel = pdata[c]["sel"]
        out[sel] = results[c]["p_out"][0, : sel.size]
    return out


_BUILD_CACHE = {}


def kernel(**inputs):
    cfg = REAL
    in_maps, dims, pdata = prepare(cfg, inputs)
    key = (tuple(dims["c_r"]), dims["pmax"], tuple(dims["bmax_r"]))
    if key not in _BUILD_CACHE:
        _BUILD_CACHE[key] = build(cfg, dims)
    nc = _BUILD_CACHE[key]
    res = _run_spmd(nc, in_maps)
    return unshard(cfg, res.results, pdata)
